# revision 55
# baseline (speedup 1.0000x reference)
"""Causal self-attention (B=2, T=2048, C=1024, H=16) on 8 trn2 NeuronCores.

Sharding: core c = (batch b = c // 4, head-group g = c % 4). Each core
computes, for its batch, QKV for heads [4g, 4g+4), causal attention, and a
partial output projection through rows [256g, 256g+256) of W_proj. The host
sums the 4 partial projections per batch (tensor-parallel unshard) and adds
b_proj.

Per-core kernel structure (all matmul inputs bf16, fp32 PSUM):
  - qk^T is produced transposed ([channel, t]) so attention scores need no
    input transposes (contraction over d=64 sits on the partition axis).
  - Scores are computed TRANSPOSED (S^T[k, q] tiles): exp(S^T) is directly
    the P^T operand the PV matmul needs. The two heads of a pair go to the
    two halves of a [128, 2, 512] PSUM tile (one bank per head) via
    row-packed K=64 matmuls at tile_position rows 0/64.
  - Causal masking costs NO cross-engine sync: for diagonal k-blocks a
    third matmul accumulates identity.T @ (-30000 * tril_mask) into the
    live 128-column window of the score PSUM (start=False), so exp()
    produces exact zeros there. GpSimd is completely idle.
  - Diagonal k-block tiles are column-restricted to the causally-live
    query range (scores, exp and PV all skip the dead columns).
  - One ACT exp per (pair, k-block) covers both heads ([128, 2, live]).
  - V carries an appended ones column (lhsT [128, 65]) so the PV matmul
    accumulates the softmax denominator as row 64 of y^T_aug for free.
  - Normalization: PSUM->SBUF copy of y_aug (bf16, frees the accumulator
    banks), then the denominator row is broadcast across partitions with a
    K=1 ones-matmul on the PE (a DMA broadcast head-blocks in-order DGE /
    DVE queues; DVE cannot read nonzero input partition offsets), then a
    DVE reciprocal from PSUM and per-head multiplies split across gpsimd
    and DVE. The whole chain is deferred into later attention steps as
    fillers; no DMA anywhere in it.
  - y lands transposed ([d, q]), exactly the lhsT the projection needs.
    Projection partials are written to DRAM in bf16 (host sums in fp32).
  - PE is the bottleneck engine (~278K matmul columns); QKV/V/projection
    matmul groups are emitted as fillers BETWEEN attention steps, weighted
    toward the early (PE-starved ACT) chunks for QKV/V and the late
    (ACT-bound) chunks for projection, so the PE instruction queue never
    drains (keeps the PE clock in its fast DVFS state).
"""

import sys
from collections import deque

for _p in ("/opt/trn_rl_repo",):
    if _p not in sys.path:
        sys.path.insert(0, _p)

import numpy as np
import ml_dtypes

import concourse.bass as bass
import concourse.tile as tile
from concourse import bacc, mybir
from concourse.bass_utils import run_bass_kernel_spmd

BF16 = mybir.dt.bfloat16
F32 = mybir.dt.float32
NP_BF16 = ml_dtypes.bfloat16

B, T, C = 2, 2048, 1024
H, D = 16, 64
N_CORES = 8
CT = C // 128   # 8 contraction tiles
TQ = T // 128   # 16 key blocks
QC = T // 512   # 4 query chunks
SCALE = 1.0 / np.sqrt(D)
NEG = -240.0  # masked-score bias; exp(SCALE*(S+NEG)) <= e^-24 ~ 4e-11
              # (kept small so the HW exp table input stays in-domain)

_compiled = None


def _build_nc(dbg=False):
    nc = bacc.Bacc("TRN2", target_bir_lowering=False, debug=False,
                   enable_asserts=False)
    if dbg:
        dbg_qkT = nc.dram_tensor("dbg_qkT", [128, 4, T], BF16, kind="ExternalOutput")
        dbg_v = nc.dram_tensor("dbg_v", [128, TQ, 4, 65], BF16, kind="ExternalOutput")
        dbg_yT = nc.dram_tensor("dbg_yT", [128, 2, T], BF16, kind="ExternalOutput")

    # pre-swizzled on host so every load is contiguous per partition
    xT_d = nc.dram_tensor("xT", [QC, 128, CT, 512], BF16, kind="ExternalInput")
    wqk_d = nc.dram_tensor("wqk", [128, 4, CT, 128], BF16, kind="ExternalInput")
    wv_d = nc.dram_tensor("wv", [128, CT, 256], BF16, kind="ExternalInput")
    wp_d = nc.dram_tensor("wp", [128, 2, C], BF16, kind="ExternalInput")
    bqk_d = nc.dram_tensor("bqk", [128, 4], F32, kind="ExternalInput")
    bv_d = nc.dram_tensor("bv", [128, 256], BF16, kind="ExternalInput")
    idn_d = nc.dram_tensor("idn", [128, 128], BF16, kind="ExternalInput")
    mskb_d = nc.dram_tensor("mskb", [128, 128], BF16, kind="ExternalInput")
    out_d = nc.dram_tensor("out", [T, C], BF16, kind="ExternalOutput")

    Exp = mybir.ActivationFunctionType.Exp

    with tile.TileContext(nc) as tc:
        with (
            tc.tile_pool(name="const", bufs=1) as cpool,
            tc.tile_pool(name="qkT", bufs=1) as qkpool,
            tc.tile_pool(name="vbuf", bufs=1) as vpool,
            tc.tile_pool(name="ybuf", bufs=1) as ypool,
            tc.tile_pool(name="pt", bufs=4) as ptpool,
            tc.tile_pool(name="norm", bufs=2) as npool,
            tc.tile_pool(name="ostage", bufs=3) as opool,
            tc.tile_pool(name="mmps", bufs=2, space="PSUM") as mmps,
            tc.tile_pool(name="sps", bufs=2, space="PSUM") as sps,
            tc.tile_pool(name="accps", bufs=2, space="PSUM") as accps,
        ):
            # ---- constants / weights ----
            # xT_s is chunk-major so chunk DMAs land contiguously; wqk_s is
            # j-major so each j-block is one contiguous DMA.
            xT_s = cpool.tile([128, QC, CT, 512], BF16)
            wqk_s = cpool.tile([128, 4, CT, 128], BF16)
            wv_s = cpool.tile([128, CT, 256], BF16)
            wp_s = cpool.tile([128, 2, C], BF16)
            bqk_s = cpool.tile([128, 4], F32)
            bv_s = cpool.tile([128, 256], BF16)
            idn_s = cpool.tile([128, 128], BF16)
            mskb_s = cpool.tile([128, 128], BF16)
            ones_s = cpool.tile([128, 64], BF16)
            nc.vector.memset(ones_s[:], 1.0)

            warm = cpool.tile([128, 1], F32)
            nc.vector.memset(warm[:], 0.0)
            nc.scalar.activation(warm[:], warm[:], Exp)

            # x chunks on the sync queue; weights in parallel on the ACT HWDGE
            # queue. One big 3D-AP DMA per tensor (per-DMA issue is ~650ns).
            # a single DMA ring sustains only ~125GB/s: spread the prologue
            # loads over four queues so transfers run in parallel.
            nc.sync.dma_start(out=xT_s[:, 0], in_=xT_d.ap()[0])
            for j in (0, 2):  # pair-0's Q/K blocks first
                nc.scalar.dma_start(out=wqk_s[:, j], in_=wqk_d.ap()[:, j])
            nc.sync.dma_start(out=idn_s[:], in_=idn_d.ap()[:])
            nc.sync.dma_start(out=mskb_s[:], in_=mskb_d.ap()[:])
            nc.sync.dma_start(out=bqk_s[:], in_=bqk_d.ap()[:])
            for j in (1, 3):
                nc.scalar.dma_start(out=wqk_s[:, j], in_=wqk_d.ap()[:, j])
            nc.scalar.dma_start(out=wv_s[:], in_=wv_d.ap()[:])
            nc.scalar.dma_start(out=bv_s[:], in_=bv_d.ap()[:])
            nc.scalar.dma_start(out=wp_s[:], in_=wp_d.ap()[:])

            qkT_s = qkpool.tile([128, 4, T], BF16)
            v_s = vpool.tile([128, TQ, 4, 65], BF16)
            nc.vector.memset(v_s[:, :, :, 64:65], 1.0)
            yT_s = ypool.tile([128, 2, T], BF16)

            # ---- emission helpers (work groups used directly or as fillers) ----
            def dma_chunk(t4):
                nc.sync.dma_start(out=xT_s[:, t4], in_=xT_d.ap()[t4])

            Identity = mybir.ActivationFunctionType.Identity

            def qkv_half(j, t4, h):
                # jtile 0: Q heads {0,1}; 1: Q {2,3}; 2: K {0,1}; 3: K {2,3}
                # emitted as two filler halves (h=0 allocates, h=1 drains).
                # Early chunks drain on ACT (idle there); last chunk on DVE.
                ps = qkv_half.ps if h else mmps.tile([128, 512], F32, tag="mm")
                qkv_half.ps = ps
                for i in range(4 * h, 4 * h + 4):
                    nc.tensor.matmul(
                        ps[:],
                        wqk_s[:, j, i, :],
                        xT_s[:, t4, i, :],
                        start=(i == 0), stop=(i == CT - 1),
                    )
                if h:
                    dst = qkT_s[:, j, 512 * t4:512 * (t4 + 1)]
                    if t4 <= 2:
                        nc.scalar.activation(dst, ps[:], Identity,
                                             bias=bqk_s[:, j:j + 1])
                    else:
                        nc.vector.tensor_scalar_add(dst, ps[:], bqk_s[:, j:j + 1])

            def qkv_group(j, t4):
                qkv_half(j, t4, 0)
                qkv_half(j, t4, 1)

            def v_group(t):
                # bias lands via an identity-matmul accumulate (bv_s rows are
                # all bv), so the drain is a plain copy on ACT (early) or DVE.
                ps = mmps.tile([128, 256], F32, tag="mm")
                for i in range(CT):
                    nc.tensor.matmul(
                        ps[:],
                        xT_s[:, t // 4, i, 128 * (t % 4):128 * (t % 4 + 1)],
                        wv_s[:, i, :],
                        start=(i == 0), stop=False,
                    )
                nc.tensor.matmul(ps[:], idn_s[:], bv_s[:],
                                 start=False, stop=True)
                dst = v_s[:, t, :, 0:64]
                src = ps[:].rearrange("p (h d) -> p h d", h=4)
                if t < 12:
                    nc.scalar.copy(dst, src)
                else:
                    nc.vector.tensor_copy(dst, src)

            def proj_half(t, n, o_t):
                ps = mmps.tile([128, 512], F32, tag="mm")
                for p2 in range(2):
                    nc.tensor.matmul(
                        ps[:],
                        yT_s[:, p2, 128 * t:128 * (t + 1)],
                        wp_s[:, p2, 512 * n:512 * (n + 1)],
                        start=(p2 == 0), stop=(p2 == 1),
                    )
                dst = o_t[:, 512 * n:512 * (n + 1)]
                if t >= 12:
                    nc.scalar.copy(dst, ps[:])
                else:
                    nc.vector.tensor_copy(dst, ps[:])
                if n == 1:
                    # alternate output rings so the epilogue's back-to-back
                    # stores don't serialize on one ~125GB/s DMA ring
                    eng = nc.scalar if (t >= 8 and t % 2) else nc.sync
                    eng.dma_start(out=out_d.ap()[128 * t:128 * (t + 1), :], in_=o_t[:])

            def proj_group(t):
                o_t = opool.tile([128, C], BF16, tag="o")
                proj_half(t, 0, o_t)
                proj_half(t, 1, o_t)

            def proj_fillers(t):
                """proj group as two filler-granular halves sharing one o_t."""
                box = {}

                def h(n, t=t, box=box):
                    if n == 0:
                        box["o"] = opool.tile([128, C], BF16, tag="o", name="o_t")
                    proj_half(t, n, box["o"])

                return [lambda: h(0), lambda: h(1)]

            NODL = (9, 9)
            fillers = deque()  # (deadline (qc, p), fn)

            def emit_filler(n=1):
                for _ in range(n):
                    if fillers:
                        fillers.popleft()[1]()

            def flush_due(key):
                """Emit every queued filler whose deadline is <= key."""
                keep = deque()
                while fillers:
                    dl, fn = fillers.popleft()
                    if dl <= key:
                        fn()
                    else:
                        keep.append((dl, fn))
                fillers.extend(keep)

            # ---- prologue: pair-0's Q/K j-tiles + V t-block 0 (v1..3 are the
            # first fillers; consume(kb) only needs v_s[kb] by step kb) ----
            qkv_group(0, 0)
            qkv_group(2, 0)
            v_group(0)

            # ---- attention: S^T tiles [k-block, q-chunk], flash over k ----
            for qc in range(QC):
                # stage work for later chunks (see scheduling notes in header)
                if qc == 0:
                    for t in (1, 2, 3):
                        fillers.append(((0, 1), lambda t=t: v_group(t)))
                    fillers.append(((0, 1), lambda: qkv_group(1, 0)))
                    fillers.append(((0, 1), lambda: qkv_group(3, 0)))
                if qc + 1 < QC:
                    dl = (qc + 1, 0)
                    fillers.append((dl, lambda t4=qc + 1: dma_chunk(t4)))
                    for j in range(4):
                        fillers.append((dl, lambda j=j, t4=qc + 1: qkv_half(j, t4, 0)))
                        fillers.append((dl, lambda j=j, t4=qc + 1: qkv_half(j, t4, 1)))
                    # V for the next chunk; the last chunk's V groups are
                    # deferred into qc3 itself (it is filler-starved).
                    if qc + 1 < QC - 1:
                        for t in range(4 * (qc + 1), 4 * (qc + 2)):
                            fillers.append((dl, lambda t=t: v_group(t)))
                if qc == QC - 1:
                    for t in range(4 * qc, 4 * (qc + 1)):
                        fillers.append(((qc, 1), lambda t=t: v_group(t)))
                # projection: qc0+t4,t5 during qc2; rest of qc1+qc2 during qc3
                for tp in {2: range(0, 6), 3: range(6, 12)}.get(qc, ()):
                    for f in proj_fillers(tp):
                        fillers.append((NODL, f))

                for p in range(2):
                    flush_due((qc, p))
                    jq, jk = p, 2 + p
                    nkb = 4 * qc + 4
                    ya = accps.tile([65, 512], F32, tag="acc")
                    yb = accps.tile([65, 512], F32, tag="acc")
                    pts = {}

                    def stage(kb, qc=qc, jq=jq, jk=jk, pts=None):
                        """score (+ causal bias) matmuls + exp for both heads"""
                        m = kb - 4 * qc  # >= 0 on the diagonal chunk
                        lv = 128 * max(m, 0)  # first causally-live column
                        s_ps = sps.tile([128, 2, 512], F32, tag="spair")
                        for hi in range(2):
                            nc.tensor.matmul(
                                s_ps[:, hi, lv:512],
                                qkT_s[64 * hi:64 * (hi + 1), jk, 128 * kb:128 * (kb + 1)],
                                qkT_s[64 * hi:64 * (hi + 1), jq, 512 * qc + lv:512 * (qc + 1)],
                                start=True, stop=(m < 0),
                                tile_position=(64 * hi, 0), skip_group_check=True)
                        if m >= 0:
                            for hi in range(2):
                                nc.tensor.matmul(
                                    s_ps[:, hi, lv:lv + 128],
                                    idn_s[:], mskb_s[:],
                                    start=False, stop=True,
                                    tile_position=(0, 0), skip_group_check=True)
                        pt = ptpool.tile([128, 2, 512], BF16, tag="pt")
                        nc.scalar.activation(pt[:, :, lv:512], s_ps[:, :, lv:512],
                                             Exp, scale=SCALE)
                        pts[kb] = (pt, lv)

                    def consume(kb, p=p, ya=ya, yb=yb, nkb=nkb, qc=qc, pts=None):
                        pt, lv = pts.pop(kb)
                        for hi, y_ps in ((0, ya), (1, yb)):
                            nc.tensor.matmul(
                                y_ps[:, lv:512],
                                v_s[:, kb, 2 * p + hi, :],
                                pt[:, hi, lv:512],
                                start=(kb == 0), stop=(kb == nkb - 1),
                                skip_group_check=True)

                    DEPTH = 2
                    for kb in range(min(DEPTH, nkb)):
                        stage(kb, pts=pts)
                    for kb in range(nkb):
                        if kb + DEPTH < nkb:
                            stage(kb + DEPTH, pts=pts)
                        consume(kb, pts=pts)
                        emit_filler(2 if qc == 0 else 1)

                    # normalize + write y^T (head A -> partitions 0:64, B -> 64:128).
                    # Copies run inline (they free the accumulator PSUM banks);
                    # reciprocal -> broadcast DMA -> multiply are deferred into
                    # the next attention steps.
                    yc = npool.tile([65, 2, 512], BF16, tag="yc")
                    nc.vector.tensor_copy(yc[:, 0, :], ya[:])
                    nc.vector.tensor_copy(yc[:, 1, :], yb[:])
                    rr = npool.tile([64, 2, 512], F32, tag="rr")
                    dps = {}

                    def norm_bcast(yc=yc, dps=dps):
                        # broadcast the partition-64 denominator row across 64
                        # partitions with a K=1 ones-matmul (no DMA, no DVE
                        # partition-offset reads -- both are unreliable/slow).
                        for hi in range(2):
                            dp = mmps.tile([128, 512], F32, tag="mm", name="dps")
                            nc.tensor.matmul(
                                dp[0:64, :], ones_s[64:65, :], yc[64:65, hi, :],
                                start=True, stop=True,
                                tile_position=(64, 0), skip_group_check=True)
                            dps[hi] = dp

                    def norm_recip(rr=rr, dps=dps):
                        for hi in range(2):
                            nc.vector.reciprocal_approx_fast(
                                rr[:, hi, :], dps.pop(hi)[0:64, :])

                    def norm_mul(p=p, qc=qc, yc=yc, rr=rr):
                        # hi=0 on gpsimd (otherwise idle), hi=1 on DVE
                        nc.gpsimd.tensor_mul(
                            yT_s[0:64, p, 512 * qc:512 * (qc + 1)],
                            yc[0:64, 0, :], rr[:, 0, :])
                        nc.vector.tensor_mul(
                            yT_s[64:128, p, 512 * qc:512 * (qc + 1)],
                            yc[0:64, 1, :], rr[:, 1, :])

                    fillers.appendleft((NODL, norm_mul))
                    fillers.appendleft((NODL, norm_recip))
                    fillers.appendleft((NODL, norm_bcast))

            # ---- epilogue: leftover fillers (incl. last norm chain) + final
            # projection chunk (ACT drains; ACT is idle here) ----
            emit_filler(len(fillers))
            if dbg:
                nc.sync.dma_start(out=dbg_qkT.ap()[:], in_=qkT_s[:])
                nc.sync.dma_start(out=dbg_v.ap()[:], in_=v_s[:])
                nc.sync.dma_start(out=dbg_yT.ap()[:], in_=yT_s[:])
            for t in range(4 * (QC - 1), TQ):
                proj_group(t)

    nc.compile()
    return nc


def _shard_inputs(x, W_attn, b_attn, W_proj, b_proj):
    """Build the 8 per-core input maps (numpy, bf16 where applicable)."""
    pp = np.arange(128)[:, None]
    jj = np.arange(128)[None, :]
    mskb = np.where(pp > jj, NEG, 0.0).astype(NP_BF16)  # [128, 128]
    idn = np.eye(128, dtype=NP_BF16)
    in_maps = []
    for c in range(N_CORES):
        b, g = c // 4, c % 4
        ch = slice(256 * g, 256 * (g + 1))
        wq = W_attn[:, ch]
        wk = W_attn[:, C:][:, ch]
        wv = W_attn[:, 2 * C:][:, ch]
        wqk = np.concatenate([wq, wk], axis=1).astype(NP_BF16)
        # [C, 512] -> [128, 4j, CT, 128]
        wqk = np.ascontiguousarray(
            wqk.reshape(CT, 128, 4, 128).transpose(1, 2, 0, 3))
        bq = b_attn[ch]
        bk = b_attn[C:][ch]
        bv = b_attn[2 * C:][ch]
        bqk = np.concatenate([bq, bk]).reshape(4, 128).T.astype(np.float32)  # [128, 4]
        xTc = (x[b].T.reshape(C, QC, 512).transpose(1, 0, 2)
               .reshape(QC, CT, 128, 512).transpose(0, 2, 1, 3))
        wvc = wv.astype(NP_BF16).reshape(CT, 128, 256).transpose(1, 0, 2)
        wpc = (W_proj[ch, :].astype(NP_BF16)
               .reshape(2, 128, C).transpose(1, 0, 2))
        in_maps.append({
            "xT": np.ascontiguousarray(xTc).astype(NP_BF16),
            "wqk": wqk,
            "wv": np.ascontiguousarray(wvc),
            "wp": np.ascontiguousarray(wpc),
            "bqk": np.ascontiguousarray(bqk),
            "bv": np.broadcast_to(bv.astype(NP_BF16), (128, 256)).copy(),
            "idn": idn,
            "mskb": mskb,
        })
    return in_maps


def _run(in_maps, trace=False, **kw):
    global _compiled
    if _compiled is None:
        _compiled = _build_nc()
    return run_bass_kernel_spmd(_compiled, in_maps, list(range(N_CORES)),
                                trace=trace, **kw)


def kernel(x, W_attn, b_attn, W_proj, b_proj):
    x = np.asarray(x, dtype=np.float32)
    W_attn = np.asarray(W_attn, dtype=np.float32)
    b_attn = np.asarray(b_attn, dtype=np.float32)
    W_proj = np.asarray(W_proj, dtype=np.float32)
    b_proj = np.asarray(b_proj, dtype=np.float32)

    in_maps = _shard_inputs(x, W_attn, b_attn, W_proj, b_proj)
    res = _run(in_maps)
    out = np.zeros((B, T, C), dtype=np.float32)
    for c in range(N_CORES):
        out[c // 4] += np.asarray(res.results[c]["out"], dtype=np.float32)
    out += b_proj
    return out


# revision 57
# speedup vs baseline: 1.0350x; 1.0350x over previous
"""Causal self-attention (B=2, T=2048, C=1024, H=16) on 8 trn2 NeuronCores.

Sharding: core c = (batch b = c // 4, head-group g = c % 4). Each core
computes, for its batch, QKV for heads [4g, 4g+4), causal attention, and a
partial output projection through rows [256g, 256g+256) of W_proj. The host
sums the 4 partial projections per batch (tensor-parallel unshard) and adds
b_proj.

Per-core kernel structure (all matmul inputs bf16, fp32 PSUM):
  - qk^T is produced transposed ([channel, t]) so attention scores need no
    input transposes (contraction over d=64 sits on the partition axis).
  - Scores are computed TRANSPOSED (S^T[k, q] tiles): exp(S^T) is directly
    the P^T operand the PV matmul needs. The two heads of a pair go to the
    two halves of a [128, 2, 512] PSUM tile (one bank per head) via
    row-packed K=64 matmuls at tile_position rows 0/64.
  - Causal masking costs NO cross-engine sync: for diagonal k-blocks a
    third matmul accumulates identity.T @ (-30000 * tril_mask) into the
    live 128-column window of the score PSUM (start=False), so exp()
    produces exact zeros there. GpSimd is completely idle.
  - Diagonal k-block tiles are column-restricted to the causally-live
    query range (scores, exp and PV all skip the dead columns).
  - One ACT exp per (pair, k-block) covers both heads ([128, 2, live]).
  - V carries an appended ones column (lhsT [128, 65]) so the PV matmul
    accumulates the softmax denominator as row 64 of y^T_aug for free.
  - Normalization: PSUM->SBUF copy of y_aug (bf16, frees the accumulator
    banks), then the denominator row is broadcast across partitions with a
    K=1 ones-matmul on the PE (a DMA broadcast head-blocks in-order DGE /
    DVE queues; DVE cannot read nonzero input partition offsets), then a
    DVE reciprocal from PSUM and per-head multiplies split across gpsimd
    and DVE. The whole chain is deferred into later attention steps as
    fillers; no DMA anywhere in it.
  - y lands transposed ([d, q]), exactly the lhsT the projection needs.
    Projection partials are written to DRAM in bf16 (host sums in fp32).
  - PE is the bottleneck engine (~278K matmul columns); QKV/V/projection
    matmul groups are emitted as fillers BETWEEN attention steps, weighted
    toward the early (PE-starved ACT) chunks for QKV/V and the late
    (ACT-bound) chunks for projection, so the PE instruction queue never
    drains (keeps the PE clock in its fast DVFS state).
"""

import sys
from collections import deque

for _p in ("/opt/trn_rl_repo",):
    if _p not in sys.path:
        sys.path.insert(0, _p)

import numpy as np
import ml_dtypes

import concourse.bass as bass
import concourse.tile as tile
from concourse import bacc, mybir
from concourse.bass_utils import run_bass_kernel_spmd

BF16 = mybir.dt.bfloat16
F32 = mybir.dt.float32
NP_BF16 = ml_dtypes.bfloat16

B, T, C = 2, 2048, 1024
H, D = 16, 64
N_CORES = 8
CT = C // 128   # 8 contraction tiles
TQ = T // 128   # 16 key blocks
QC = T // 512   # 4 query chunks
SCALE = 1.0 / np.sqrt(D)
NEG = -240.0  # masked-score bias; exp(SCALE*(S+NEG)) <= e^-24 ~ 4e-11
              # (kept small so the HW exp table input stays in-domain)

_compiled = None


def _build_nc(dbg=False):
    nc = bacc.Bacc("TRN2", target_bir_lowering=False, debug=False,
                   enable_asserts=False)
    if dbg:
        dbg_qkT = nc.dram_tensor("dbg_qkT", [128, 4, T], BF16, kind="ExternalOutput")
        dbg_v = nc.dram_tensor("dbg_v", [128, TQ, 4, 65], BF16, kind="ExternalOutput")
        dbg_yT = nc.dram_tensor("dbg_yT", [128, 2, T], BF16, kind="ExternalOutput")

    # pre-swizzled on host so every load is contiguous per partition
    xT_d = nc.dram_tensor("xT", [QC, 128, CT, 512], BF16, kind="ExternalInput")
    wqk_d = nc.dram_tensor("wqk", [128, 4, CT, 128], BF16, kind="ExternalInput")
    wv_d = nc.dram_tensor("wv", [128, CT, 256], BF16, kind="ExternalInput")
    wp_d = nc.dram_tensor("wp", [128, 2, C], BF16, kind="ExternalInput")
    bqk_d = nc.dram_tensor("bqk", [128, 4], F32, kind="ExternalInput")
    bv_d = nc.dram_tensor("bv", [128, 256], BF16, kind="ExternalInput")
    idn_d = nc.dram_tensor("idn", [128, 128], BF16, kind="ExternalInput")
    mskb_d = nc.dram_tensor("mskb", [128, 128], BF16, kind="ExternalInput")
    out_d = nc.dram_tensor("out", [T, C], BF16, kind="ExternalOutput")

    Exp = mybir.ActivationFunctionType.Exp

    with tile.TileContext(nc) as tc:
        with (
            tc.tile_pool(name="const", bufs=1) as cpool,
            tc.tile_pool(name="qkT", bufs=1) as qkpool,
            tc.tile_pool(name="vbuf", bufs=1) as vpool,
            tc.tile_pool(name="ybuf", bufs=1) as ypool,
            tc.tile_pool(name="pt", bufs=4) as ptpool,
            tc.tile_pool(name="norm", bufs=2) as npool,
            tc.tile_pool(name="ostage", bufs=3) as opool,
            tc.tile_pool(name="mmps", bufs=2, space="PSUM") as mmps,
            tc.tile_pool(name="sps", bufs=2, space="PSUM") as sps,
            tc.tile_pool(name="accps", bufs=2, space="PSUM") as accps,
        ):
            # ---- constants / weights ----
            # xT_s is chunk-major so chunk DMAs land contiguously; wqk_s is
            # j-major so each j-block is one contiguous DMA.
            xT_s = cpool.tile([128, QC, CT, 512], BF16)
            wqk_s = cpool.tile([128, 4, CT, 128], BF16)
            wv_s = cpool.tile([128, CT, 256], BF16)
            wp_s = cpool.tile([128, 2, C], BF16)
            bqk_s = cpool.tile([128, 4], F32)
            bv_s = cpool.tile([128, 256], BF16)
            idn_s = cpool.tile([128, 128], BF16)
            mskb_s = cpool.tile([128, 128], BF16)
            ones_s = cpool.tile([128, 64], BF16)
            nc.vector.memset(ones_s[:], 1.0)

            warm = cpool.tile([128, 1], F32)
            nc.vector.memset(warm[:], 0.0)
            nc.scalar.activation(warm[:], warm[:], Exp)

            # x chunks on the sync queue; weights in parallel on the ACT HWDGE
            # queue. One big 3D-AP DMA per tensor (per-DMA issue is ~650ns).
            # a single DMA ring sustains only ~125GB/s: spread the prologue
            # loads over four queues so transfers run in parallel.
            nc.sync.dma_start(out=xT_s[:, 0], in_=xT_d.ap()[0])
            for j in (0, 2):  # pair-0's Q/K blocks first
                nc.scalar.dma_start(out=wqk_s[:, j], in_=wqk_d.ap()[:, j])
            nc.sync.dma_start(out=idn_s[:], in_=idn_d.ap()[:])
            nc.sync.dma_start(out=mskb_s[:], in_=mskb_d.ap()[:])
            nc.sync.dma_start(out=bqk_s[:], in_=bqk_d.ap()[:])
            for j in (1, 3):
                nc.scalar.dma_start(out=wqk_s[:, j], in_=wqk_d.ap()[:, j])
            nc.scalar.dma_start(out=wv_s[:], in_=wv_d.ap()[:])
            nc.scalar.dma_start(out=bv_s[:], in_=bv_d.ap()[:])
            nc.scalar.dma_start(out=wp_s[:], in_=wp_d.ap()[:])

            qkT_s = qkpool.tile([128, 4, T], BF16)
            v_s = vpool.tile([128, TQ, 4, 65], BF16)
            nc.vector.memset(v_s[:, :, :, 64:65], 1.0)
            yT_s = ypool.tile([128, 2, T], BF16)

            # ---- emission helpers (work groups used directly or as fillers) ----
            def dma_chunk(t4):
                nc.sync.dma_start(out=xT_s[:, t4], in_=xT_d.ap()[t4])

            Identity = mybir.ActivationFunctionType.Identity

            def qkv_half(j, t4, h):
                # jtile 0: Q heads {0,1}; 1: Q {2,3}; 2: K {0,1}; 3: K {2,3}
                # emitted as two filler halves (h=0 allocates, h=1 drains).
                # Early chunks drain on ACT (idle there); last chunk on DVE.
                ps = qkv_half.ps if h else mmps.tile([128, 512], F32, tag="mm")
                qkv_half.ps = ps
                for i in range(4 * h, 4 * h + 4):
                    nc.tensor.matmul(
                        ps[:],
                        wqk_s[:, j, i, :],
                        xT_s[:, t4, i, :],
                        start=(i == 0), stop=(i == CT - 1),
                    )
                if h:
                    dst = qkT_s[:, j, 512 * t4:512 * (t4 + 1)]
                    if t4 <= 2:
                        nc.scalar.activation(dst, ps[:], Identity,
                                             bias=bqk_s[:, j:j + 1])
                    else:
                        nc.vector.tensor_scalar_add(dst, ps[:], bqk_s[:, j:j + 1])

            def qkv_group(j, t4):
                qkv_half(j, t4, 0)
                qkv_half(j, t4, 1)

            def v_group(t):
                # bias lands via an identity-matmul accumulate (bv_s rows are
                # all bv), so the drain is a plain copy on ACT (early) or DVE.
                ps = mmps.tile([128, 256], F32, tag="mm")
                for i in range(CT):
                    nc.tensor.matmul(
                        ps[:],
                        xT_s[:, t // 4, i, 128 * (t % 4):128 * (t % 4 + 1)],
                        wv_s[:, i, :],
                        start=(i == 0), stop=False,
                    )
                nc.tensor.matmul(ps[:], idn_s[:], bv_s[:],
                                 start=False, stop=True)
                dst = v_s[:, t, :, 0:64]
                src = ps[:].rearrange("p (h d) -> p h d", h=4)
                if t < 12:
                    nc.scalar.copy(dst, src)
                else:
                    nc.vector.tensor_copy(dst, src)

            def proj_half(t, n, o_t):
                ps = mmps.tile([128, 512], F32, tag="mm")
                for p2 in range(2):
                    nc.tensor.matmul(
                        ps[:],
                        yT_s[:, p2, 128 * t:128 * (t + 1)],
                        wp_s[:, p2, 512 * n:512 * (n + 1)],
                        start=(p2 == 0), stop=(p2 == 1),
                    )
                dst = o_t[:, 512 * n:512 * (n + 1)]
                if t >= 12:
                    nc.scalar.copy(dst, ps[:])
                else:
                    nc.vector.tensor_copy(dst, ps[:])
                if n == 1:
                    # alternate output rings so the epilogue's back-to-back
                    # stores don't serialize on one ~125GB/s DMA ring
                    eng = nc.scalar if (t >= 8 and t % 2) else nc.sync
                    eng.dma_start(out=out_d.ap()[128 * t:128 * (t + 1), :], in_=o_t[:])

            def proj_group(t):
                o_t = opool.tile([128, C], BF16, tag="o")
                proj_half(t, 0, o_t)
                proj_half(t, 1, o_t)

            def proj_fillers(t):
                """proj group as two filler-granular halves sharing one o_t."""
                box = {}

                def h(n, t=t, box=box):
                    if n == 0:
                        box["o"] = opool.tile([128, C], BF16, tag="o", name="o_t")
                    proj_half(t, n, box["o"])

                return [lambda: h(0), lambda: h(1)]

            NODL = (9, 9)
            fillers = deque()  # (deadline (qc, p), fn)

            def emit_filler(n=1):
                for _ in range(n):
                    if fillers:
                        fillers.popleft()[1]()

            def flush_due(key):
                """Emit every queued filler whose deadline is <= key."""
                keep = deque()
                while fillers:
                    dl, fn = fillers.popleft()
                    if dl <= key:
                        fn()
                    else:
                        keep.append((dl, fn))
                fillers.extend(keep)

            # ---- prologue: pair-0's Q/K j-tiles + V t-block 0 (v1..3 are the
            # first fillers; consume(kb) only needs v_s[kb] by step kb) ----
            qkv_group(0, 0)
            qkv_group(2, 0)
            v_group(0)

            # ---- attention: S^T tiles [k-block, q-chunk], flash over k ----
            for qc in range(QC):
                # stage work for later chunks (see scheduling notes in header)
                if qc == 0:
                    for t in (1, 2, 3):
                        fillers.append(((0, 1), lambda t=t: v_group(t)))
                    fillers.append(((0, 1), lambda: qkv_group(1, 0)))
                    fillers.append(((0, 1), lambda: qkv_group(3, 0)))
                if qc + 1 < QC:
                    dl = (qc + 1, 0)
                    fillers.append((dl, lambda t4=qc + 1: dma_chunk(t4)))
                    for j in range(4):
                        fillers.append((dl, lambda j=j, t4=qc + 1: qkv_half(j, t4, 0)))
                        fillers.append((dl, lambda j=j, t4=qc + 1: qkv_half(j, t4, 1)))
                    # V for the next chunk; the last chunk's V groups are
                    # deferred into qc3 itself (it is filler-starved).
                    if qc + 1 < QC - 1:
                        for t in range(4 * (qc + 1), 4 * (qc + 2)):
                            fillers.append((dl, lambda t=t: v_group(t)))
                if qc == QC - 1:
                    for t in range(4 * qc, 4 * (qc + 1)):
                        fillers.append(((qc, 1), lambda t=t: v_group(t)))
                # projection: qc0+t4,t5 during qc2; rest of qc1+qc2 during qc3
                for tp in {2: range(0, 6), 3: range(6, 12)}.get(qc, ()):
                    for f in proj_fillers(tp):
                        fillers.append((NODL, f))

                for p in range(2):
                    flush_due((qc, p))
                    jq, jk = p, 2 + p
                    nkb = 4 * qc + 4
                    ya = accps.tile([65, 512], F32, tag="acc")
                    yb = accps.tile([65, 512], F32, tag="acc")
                    pts = {}

                    def stage(kb, qc=qc, jq=jq, jk=jk, pts=None):
                        """score (+ causal bias) matmuls + exp for both heads"""
                        m = kb - 4 * qc  # >= 0 on the diagonal chunk
                        lv = 128 * max(m, 0)  # first causally-live column
                        s_ps = sps.tile([128, 2, 512], F32, tag="spair")
                        for hi in range(2):
                            nc.tensor.matmul(
                                s_ps[:, hi, lv:512],
                                qkT_s[64 * hi:64 * (hi + 1), jk, 128 * kb:128 * (kb + 1)],
                                qkT_s[64 * hi:64 * (hi + 1), jq, 512 * qc + lv:512 * (qc + 1)],
                                start=True, stop=(m < 0),
                                tile_position=(64 * hi, 0), skip_group_check=True)
                        if m >= 0:
                            for hi in range(2):
                                nc.tensor.matmul(
                                    s_ps[:, hi, lv:lv + 128],
                                    idn_s[:], mskb_s[:],
                                    start=False, stop=True,
                                    tile_position=(0, 0), skip_group_check=True)
                        pt = ptpool.tile([128, 2, 512], BF16, tag="pt")
                        nc.scalar.activation(pt[:, :, lv:512], s_ps[:, :, lv:512],
                                             Exp, scale=SCALE)
                        pts[kb] = (pt, lv)

                    def consume(kb, p=p, ya=ya, yb=yb, nkb=nkb, qc=qc, pts=None):
                        pt, lv = pts.pop(kb)
                        for hi, y_ps in ((0, ya), (1, yb)):
                            nc.tensor.matmul(
                                y_ps[:, lv:512],
                                v_s[:, kb, 2 * p + hi, :],
                                pt[:, hi, lv:512],
                                start=(kb == 0), stop=(kb == nkb - 1),
                                skip_group_check=True)

                    DEPTH = 2
                    for kb in range(min(DEPTH, nkb)):
                        stage(kb, pts=pts)
                    for kb in range(nkb):
                        if kb + DEPTH < nkb:
                            stage(kb + DEPTH, pts=pts)
                        consume(kb, pts=pts)
                        emit_filler(2 if qc == 0 else 1)

                    # normalize + write y^T (head A -> partitions 0:64, B -> 64:128).
                    # Copies run inline (they free the accumulator PSUM banks);
                    # reciprocal -> broadcast DMA -> multiply are deferred into
                    # the next attention steps.
                    yc = npool.tile([65, 2, 512], BF16, tag="yc")
                    nc.vector.tensor_copy(yc[:, 0, :], ya[:])
                    nc.vector.tensor_copy(yc[:, 1, :], yb[:])
                    rr = npool.tile([64, 2, 512], F32, tag="rr")
                    dps = {}

                    def norm_bcast(yc=yc, dps=dps):
                        # broadcast the partition-64 denominator row across 64
                        # partitions with a K=1 ones-matmul (no DMA, no DVE
                        # partition-offset reads -- both are unreliable/slow).
                        for hi in range(2):
                            dp = mmps.tile([128, 512], F32, tag="mm", name="dps")
                            nc.tensor.matmul(
                                dp[0:64, :], ones_s[64:65, :], yc[64:65, hi, :],
                                start=True, stop=True,
                                tile_position=(64, 0), skip_group_check=True)
                            dps[hi] = dp

                    def norm_recip(rr=rr, dps=dps):
                        for hi in range(2):
                            nc.vector.reciprocal_approx_fast(
                                rr[:, hi, :], dps.pop(hi)[0:64, :])

                    def norm_mul(p=p, qc=qc, yc=yc, rr=rr):
                        # hi=0 on gpsimd (otherwise idle), hi=1 on DVE
                        nc.gpsimd.tensor_mul(
                            yT_s[0:64, p, 512 * qc:512 * (qc + 1)],
                            yc[0:64, 0, :], rr[:, 0, :])
                        nc.vector.tensor_mul(
                            yT_s[64:128, p, 512 * qc:512 * (qc + 1)],
                            yc[0:64, 1, :], rr[:, 1, :])

                    # insert the norm chain AFTER the next queued filler: the
                    # bcast matmul waits on the DVE yc-copies just queued, so
                    # one slot of independent PE work absorbs that latency
                    # instead of the PE stalling on it.
                    pos = 1 if fillers else 0
                    for fn in (norm_mul, norm_recip, norm_bcast):
                        fillers.insert(pos, (NODL, fn))

            # ---- epilogue: leftover fillers (incl. last norm chain) + final
            # projection chunk (ACT drains; ACT is idle here) ----
            emit_filler(len(fillers))
            if dbg:
                nc.sync.dma_start(out=dbg_qkT.ap()[:], in_=qkT_s[:])
                nc.sync.dma_start(out=dbg_v.ap()[:], in_=v_s[:])
                nc.sync.dma_start(out=dbg_yT.ap()[:], in_=yT_s[:])
            for t in range(4 * (QC - 1), TQ):
                proj_group(t)

    nc.compile()
    return nc


def _shard_inputs(x, W_attn, b_attn, W_proj, b_proj):
    """Build the 8 per-core input maps (numpy, bf16 where applicable)."""
    pp = np.arange(128)[:, None]
    jj = np.arange(128)[None, :]
    mskb = np.where(pp > jj, NEG, 0.0).astype(NP_BF16)  # [128, 128]
    idn = np.eye(128, dtype=NP_BF16)
    in_maps = []
    for c in range(N_CORES):
        b, g = c // 4, c % 4
        ch = slice(256 * g, 256 * (g + 1))
        wq = W_attn[:, ch]
        wk = W_attn[:, C:][:, ch]
        wv = W_attn[:, 2 * C:][:, ch]
        wqk = np.concatenate([wq, wk], axis=1).astype(NP_BF16)
        # [C, 512] -> [128, 4j, CT, 128]
        wqk = np.ascontiguousarray(
            wqk.reshape(CT, 128, 4, 128).transpose(1, 2, 0, 3))
        bq = b_attn[ch]
        bk = b_attn[C:][ch]
        bv = b_attn[2 * C:][ch]
        bqk = np.concatenate([bq, bk]).reshape(4, 128).T.astype(np.float32)  # [128, 4]
        xTc = (x[b].T.reshape(C, QC, 512).transpose(1, 0, 2)
               .reshape(QC, CT, 128, 512).transpose(0, 2, 1, 3))
        wvc = wv.astype(NP_BF16).reshape(CT, 128, 256).transpose(1, 0, 2)
        wpc = (W_proj[ch, :].astype(NP_BF16)
               .reshape(2, 128, C).transpose(1, 0, 2))
        in_maps.append({
            "xT": np.ascontiguousarray(xTc).astype(NP_BF16),
            "wqk": wqk,
            "wv": np.ascontiguousarray(wvc),
            "wp": np.ascontiguousarray(wpc),
            "bqk": np.ascontiguousarray(bqk),
            "bv": np.broadcast_to(bv.astype(NP_BF16), (128, 256)).copy(),
            "idn": idn,
            "mskb": mskb,
        })
    return in_maps


def _run(in_maps, trace=False, **kw):
    global _compiled
    if _compiled is None:
        _compiled = _build_nc()
    return run_bass_kernel_spmd(_compiled, in_maps, list(range(N_CORES)),
                                trace=trace, **kw)


def kernel(x, W_attn, b_attn, W_proj, b_proj):
    x = np.asarray(x, dtype=np.float32)
    W_attn = np.asarray(W_attn, dtype=np.float32)
    b_attn = np.asarray(b_attn, dtype=np.float32)
    W_proj = np.asarray(W_proj, dtype=np.float32)
    b_proj = np.asarray(b_proj, dtype=np.float32)

    in_maps = _shard_inputs(x, W_attn, b_attn, W_proj, b_proj)
    res = _run(in_maps)
    out = np.zeros((B, T, C), dtype=np.float32)
    for c in range(N_CORES):
        out[c // 4] += np.asarray(res.results[c]["out"], dtype=np.float32)
    out += b_proj
    return out
